# revision 2
# baseline (speedup 1.0000x reference)
"""Trainium2 Bass kernel v2: batched channel-attention (Gram-matrix form).

Key changes vs baseline:
- x shipped as fp16 from host (8 MB/core HBM read instead of 16 MB)
- y written as fp16, upcast on host (8 MB/core HBM write instead of 16 MB)
- symmetric Gram: compute G00/G01/G11 only; reconstruct G10 = G01^T
- grouped transpose->copy->matmul pipeline (4 subtiles per group) to cut
  per-instruction overhead on the copy engines
- one ones-memset per group instead of per subtile
"""

import bisect
from contextlib import ExitStack

import concourse.bass as bass
import concourse.tile as tile
from concourse import bacc, mybir
from concourse.masks import make_identity

F32 = mybir.dt.float32
F32R = mybir.dt.float32r
F16 = mybir.dt.float16

C = 256
CH = 128  # half of C, = partition count


def build_nc(
    N=16384,
    chunks=(512, 512, 1024, 2048, 2048, 2048, 4096, 2048, 2048),
    out_chunks=(512, 512, 1024, 2048, 2048, 2048, 2048, 2048, 2048, 1024, 512, 512),
    nt=512,
    group=4,
    tpsum_bufs=4,
    xt_bufs=8,
    attv_bufs=4,
    out_bufs=3,
    avw=1024,
    xbar_mod=0,      # groups with g % xbar_mod == 1 transpose via DMA xbar
                     # (xbar works single-core but corrupts + serializes
                     #  when all 8 cores run it concurrently - keep 0)
    alg_f32r=True,   # run the CxC algebra matmuls in fp32r (1 cyc/row)
    warm_start=15,   # dummy PE transposes at t0: ramp the p-state before
                     # the real stream arrives (cold PE runs at 0.65GHz)
    warm_mid=6,      # dummies before phase B to hold the p-state through
                     # the softmax serial chain
    copy_mod=3,      # 1 of copy_mod phase-A copies goes to ACT, rest DVE
):
    NSUBS = N // 128
    NGROUPS = NSUBS // group
    assert sum(chunks) == N
    assert all(c % (128 * group) == 0 for c in chunks)
    nc = bacc.Bacc(None, target_bir_lowering=False)

    # fp32r is bit-identical to fp32 in memory; declaring algebra inputs as
    # fp32r keeps the fp32r-producer check happy while the PE runs the CxC
    # algebra at 1 cycle/row instead of fp32's 4.
    AD = F32R if alg_f32r else F32

    x = nc.dram_tensor("x", [C, N], F16, kind="ExternalInput")
    w1t = nc.dram_tensor("w1t", [C, C], AD, kind="ExternalInput")
    w2t = nc.dram_tensor("w2t", [C, C], AD, kind="ExternalInput")
    b1 = nc.dram_tensor("b1", [1, C], AD, kind="ExternalInput")
    b2 = nc.dram_tensor("b2", [1, C], AD, kind="ExternalInput")
    y = nc.dram_tensor("y", [C, N], F16, kind="ExternalOutput")

    def f(ap):
        """plain-f32 view of an fp32r buffer for non-matmul consumers"""
        return ap.bitcast(F32) if ap.dtype == F32R else ap

    starts = []
    pos = 0
    for w in chunks:
        starts.append(pos)
        pos += w

    dma_engines = [nc.sync, nc.scalar]

    def copy_on(idx, out, in_):
        """PSUM->SBUF evacuation copy: mostly DVE, 1-in-copy_mod on ACT
        (ACT is ~1.6x slower per copy and also runs exp + phase-B evac)."""
        if copy_mod > 0 and idx % copy_mod == copy_mod - 1:
            nc.scalar.copy(out, in_)
        else:
            nc.vector.tensor_copy(out, in_)

    with tile.TileContext(nc) as tc, ExitStack() as ctx:
        consts = ctx.enter_context(tc.tile_pool(name="consts", bufs=1))
        xfp = ctx.enter_context(tc.tile_pool(name="xf", bufs=1))
        small = ctx.enter_context(tc.tile_pool(name="small", bufs=1))

        # Load engine split: sync (HWDGE) takes the h=0 half, gpsimd (SWDGE)
        # the h=1 half, w/b go to scalar (4 quick issues, done before the
        # first copies need the ACT sequencer). DVE/ACT stay free for
        # compute: a sequencer issues its instructions in-order, so loads
        # queued there would block phase-A copies behind ~25us of issues.
        # x chunks go FIRST in each queue - every DMA issue ahead of them
        # delays the PE pipeline start by ~0.65us.
        ident = consts.tile([128, 128], F16, name="ident", tag="ident")
        make_identity(nc, ident[:])

        xfc = [[None] * len(chunks) for _ in range(2)]
        for j, w in enumerate(chunks):
            sl = slice(starts[j], starts[j] + w)
            for h in range(2):
                t = xfp.tile([CH, w], F16, name=f"xf{h}_{j}", tag=f"xf{h}_{j}")
                xfc[h][j] = t
                eng = nc.sync if h == 0 else nc.gpsimd
                eng.dma_start(t[:], x[h * CH:(h + 1) * CH, sl])

        w1_sb = [consts.tile([CH, C], AD, name=f"w1_{h}", tag=f"w1_{h}") for h in range(2)]
        w2_sb = [consts.tile([CH, C], AD, name=f"w2_{h}", tag=f"w2_{h}") for h in range(2)]
        b1_row = small.tile([1, C], AD, name="b1r", tag="b1r")
        b2_row = small.tile([1, C], AD, name="b2r", tag="b2r")
        for h in range(2):
            nc.scalar.dma_start(w1_sb[h][:], w1t[h * CH:(h + 1) * CH, :])
            nc.scalar.dma_start(w2_sb[h][:], w2t[h * CH:(h + 1) * CH, :])
        nc.scalar.dma_start(b1_row[:], b1[:])
        nc.scalar.dma_start(b2_row[:], b2[:])

        ident_f = consts.tile([128, 128], F32, name="ident_f", tag="ident_f")
        make_identity(nc, ident_f[:])

        # p-state warm-up: dummy transposes with no data deps keep the PE
        # busy (and its clock ramping) while the first x chunks are still
        # in flight; a cold PE runs at 0.65GHz, warm at 2.4GHz.
        warm_ctx = ExitStack()
        warm = warm_ctx.enter_context(tc.tile_pool(name="warm", bufs=1, space="PSUM"))
        wt = warm.tile([128, 128], F16, name="wt", tag="wt")
        for _ in range(warm_start):
            nc.tensor.transpose(wt[:], ident[:], ident[:])

        def xf_slice(h, lo, width):
            """AP for xf[h][:, lo:lo+width]; must lie inside one chunk."""
            j = bisect.bisect_right(starts, lo) - 1
            off = lo - starts[j]
            assert off + width <= chunks[j], (lo, width, j)
            return xfc[h][j][:, off:off + width]

        # ---- Phase A: G = xf xf^T (+ s columns), exploiting symmetry ----
        # g_ps[0][:, 0:258] = rows of [G00 | G01 | s0 s0]
        # g_ps[1][:, 0:130] = rows of [G11 | s1 s1]; G10 rebuilt as G01^T.
        g_sb = [small.tile([CH, C + 2], AD, name=f"gsb{h}", tag=f"gsb{h}") for h in range(2)]
        with tc.tile_pool(name="psum_g", bufs=1, space="PSUM") as pg:
            g_ps0 = pg.tile([CH, C + 2], F32, name="g0", tag="g0")
            g_ps1 = pg.tile([CH, CH + 2], F32, name="g1", tag="g1")
            with tc.tile_pool(name="psum_t", bufs=tpsum_bufs, space="PSUM") as pt, \
                 tc.tile_pool(name="xt", bufs=xt_bufs) as xt_pool:

                ncopy = 0
                for g in range(NGROUPS):
                    ns0 = g * group
                    use_xbar = xbar_mod > 0 and (g % xbar_mod == 1)
                    if use_xbar:
                        # transpose via the DMA xbar (SBUF->SBUF, no PSUM,
                        # no evacuation copy); 288 stride keeps partition
                        # lines 64B-aligned for the xbar tile writes
                        xts = xt_pool.tile(
                            [128, group, 288], F16, name="xtp", tag="xtp"
                        )
                        for h in range(2):
                            dma_engines[h].dma_start(
                                xts[:, :, h * CH:(h + 1) * CH],
                                xf_slice(h, ns0 * 128, group * 128),
                                transpose=True,
                            )
                        nc.vector.memset(xts[:, :, C:C + 2], 1.0)
                    else:
                        tp = pt.tile([128, group, C], F16, name="tps", tag="tps")
                        for k in range(group):
                            for h in range(2):
                                nc.tensor.transpose(
                                    tp[:, k, h * CH:(h + 1) * CH],
                                    xf_slice(h, (ns0 + k) * 128, 128),
                                    ident[:],
                                )
                        xts = xt_pool.tile(
                            [128, group, C + 2], F16, name="xts", tag="xts"
                        )
                        nc.vector.memset(xts[:, :, C:C + 2], 1.0)
                        copy_on(ncopy, xts[:, :, 0:C], tp[:])
                        ncopy += 1
                    for k in range(group):
                        first = (g == 0 and k == 0)
                        last = (g == NGROUPS - 1 and k == group - 1)
                        nc.tensor.matmul(
                            g_ps0[:],
                            xts[:, k, 0:CH],
                            xts[:, k, 0:C + 2],
                            start=first,
                            stop=last,
                        )
                        nc.tensor.matmul(
                            g_ps1[:],
                            xts[:, k, CH:C],
                            xts[:, k, CH:C + 2],
                            start=first,
                            stop=last,
                        )

            nc.vector.tensor_copy(g_sb[0][:], g_ps0[:])
            nc.scalar.copy(g_sb[1][:, CH:C + 2], g_ps1[:])

        # reconstruct G10 = G01^T
        with tc.tile_pool(name="psum_gt", bufs=1, space="PSUM") as pgt:
            gt_ps = pgt.tile([CH, CH], F32, name="gt", tag="gt")
            nc.tensor.transpose(gt_ps[:], f(g_sb[0][:, CH:C]), ident_f[:])
            nc.vector.tensor_copy(g_sb[1][:, 0:CH], gt_ps[:])

        # ---- C x C algebra ----
        # att = W1 G W2^T + (W1 s) b2^T + b1 (W2 s + N b2)^T
        with tc.tile_pool(name="psum_alg", bufs=1, space="PSUM") as pa:
            w1s_ps = pa.tile([2, C], F32, name="w1s", tag="w1s")
            w2s_ps = pa.tile([2, C], F32, name="w2s", tag="w2s")
            for h in range(2):
                nc.tensor.matmul(
                    w1s_ps[:], g_sb[h][:, C:C + 2], w1_sb[h][:],
                    start=(h == 0), stop=(h == 1),
                )
            for h in range(2):
                nc.tensor.matmul(
                    w2s_ps[:], g_sb[h][:, C:C + 2], w2_sb[h][:],
                    start=(h == 0), stop=(h == 1),
                )
            w1s_row = small.tile([1, C], AD, name="w1sr", tag="w1sr")
            w2sn_row = small.tile([1, C], AD, name="w2snr", tag="w2snr")
            nc.vector.tensor_copy(w1s_row[:], w1s_ps[0:1, :])
            # (W2 s) + N * b2
            nc.vector.scalar_tensor_tensor(
                w2sn_row[:], f(b2_row[:]), float(N), w2s_ps[0:1, :],
                op0=mybir.AluOpType.mult, op1=mybir.AluOpType.add,
            )

            u_ps = [pa.tile([CH, C], F32, name=f"u{d}", tag=f"u{d}") for d in range(2)]
            for d in range(2):
                for h in range(2):
                    nc.tensor.matmul(
                        u_ps[d][:],
                        g_sb[h][:, d * CH:(d + 1) * CH],
                        w1_sb[h][:],
                        start=(h == 0), stop=(h == 1),
                    )
            u_sb = [small.tile([CH, C], AD, name=f"usb{d}", tag=f"usb{d}") for d in range(2)]
            for d in range(2):
                nc.vector.tensor_copy(u_sb[d][:], u_ps[d][:])

            att_ps = [pa.tile([CH, C], F32, name=f"att{o}", tag=f"att{o}") for o in range(2)]
            for o in range(2):
                osl = slice(o * CH, (o + 1) * CH)
                # rank-1 terms first: their operands are ready before u_sb
                nc.tensor.matmul(
                    att_ps[o][:], w1s_row[:, osl], b2_row[:],
                    start=True, stop=False,
                )
                nc.tensor.matmul(
                    att_ps[o][:], b1_row[:, osl], w2sn_row[:],
                    start=False, stop=False,
                )
                for d in range(2):
                    nc.tensor.matmul(
                        att_ps[o][:], u_sb[d][:, osl], w2_sb[d][:],
                        start=False, stop=(d == 1),
                    )

            # ---- softmax (unnormalized exp; diag(rowsum) added so the
            # 1/rowsum scale after phase B also yields the +x residual) ----
            negmax = [small.tile([CH, 1], F32, name=f"nm{o}", tag=f"nm{o}") for o in range(2)]
            rowsum = [small.tile([CH, 1], F32, name=f"rs{o}", tag=f"rs{o}") for o in range(2)]
            rowinv = [small.tile([CH, 1], F32, name=f"ri{o}", tag=f"ri{o}") for o in range(2)]
            exp_sb = [small.tile([CH, C], F16, name=f"exp{o}", tag=f"exp{o}") for o in range(2)]
            sdiag = [small.tile([CH, CH], F32, name=f"sd{o}", tag=f"sd{o}") for o in range(2)]
            for o in range(2):
                nc.vector.reduce_max(
                    negmax[o][:], att_ps[o][:], axis=mybir.AxisListType.X,
                    negate=True,
                )
                nc.scalar.activation(
                    exp_sb[o][:], att_ps[o][:],
                    mybir.ActivationFunctionType.Exp,
                    bias=negmax[o][:], scale=1.0,
                    accum_out=rowsum[o][:],
                )
                nc.vector.reciprocal(rowinv[o][:], rowsum[o][:])
                # M = exp + diag(rowsum): y = diag(rowinv) (M @ x) = att_v + x
                osl = slice(o * CH, (o + 1) * CH)
                nc.vector.tensor_scalar_mul(
                    sdiag[o][:], ident_f[:], rowsum[o][:]
                )
                nc.vector.tensor_tensor(
                    exp_sb[o][:, osl], exp_sb[o][:, osl], sdiag[o][:],
                    op=mybir.AluOpType.add,
                )

            # ---- transpose M -> attT ----
            # atp shares att's PSUM banks (att released once exp has read it)
            attt_ps = [pa.tile([CH, C], F16, name=f"atp{d}", tag=f"att{d}") for d in range(2)]
            for d in range(2):
                for o in range(2):
                    nc.tensor.transpose(
                        attt_ps[d][:, o * CH:(o + 1) * CH],
                        exp_sb[o][:, d * CH:(d + 1) * CH],
                        ident[:],
                    )
            attt_sb = [small.tile([CH, C], F16, name=f"att_sb{d}", tag=f"att_sb{d}") for d in range(2)]
            nc.vector.tensor_copy(attt_sb[0][:], attt_ps[0][:])
            nc.scalar.copy(attt_sb[1][:], attt_ps[1][:])

        # hold the p-state through the softmax serial chain: these run
        # between the attT transposes and the first phase-B matmuls
        for _ in range(warm_mid):
            nc.tensor.transpose(wt[:], ident[:], ident[:])
        warm_ctx.close()

        # ---- Phase B: y = diag(rowinv) (M @ x), fp16 out ----
        assert sum(out_chunks) == N
        ostarts = []
        p_ = 0
        for w_ in out_chunks:
            ostarts.append(p_)
            p_ += w_
        max_oc = max(out_chunks)
        ecnt = 0
        with tc.tile_pool(name="psum_b", bufs=attv_bufs, space="PSUM") as pb, \
             tc.tile_pool(name="outp", bufs=out_bufs) as op:
            for j, oc in enumerate(out_chunks):
                for o in range(2):
                    osl = slice(o * CH, (o + 1) * CH)
                    ob = op.tile([CH, max_oc], F16, name=f"ob{o}", tag=f"ob{o}")
                    aw_full = min(oc, avw)
                    for a0 in range(0, oc, aw_full):
                        aw = min(aw_full, oc - a0)
                        av = pb.tile([CH, avw], F32, name="av", tag="av")
                        for t in range(0, aw, nt):
                            w = min(nt, aw - t)
                            lsl = slice(t, t + w)
                            for d in range(2):
                                nc.tensor.matmul(
                                    av[:, lsl],
                                    attt_sb[d][:, osl],
                                    xf_slice(d, ostarts[j] + a0 + t, w),
                                    start=(d == 0), stop=(d == 1),
                                )
                        # evac: out = av * rowinv (fp16), alternate DVE / ACT
                        if ecnt % 2 == 0:
                            nc.vector.tensor_scalar_mul(
                                ob[:, a0:a0 + aw], av[:, 0:aw], rowinv[o][:]
                            )
                        else:
                            nc.scalar.mul(
                                ob[:, a0:a0 + aw], av[:, 0:aw], rowinv[o][:]
                            )
                        ecnt += 1
                    eng = dma_engines[(2 * j + o) % 2]
                    eng.dma_start(
                        y[osl, ostarts[j]:ostarts[j] + oc], ob[:, 0:oc]
                    )

    nc.compile()
    return nc


# ---------------------------------------------------------------------------
# Host-side entry point: shard batch over the 8 NeuronCores, run, gather.
# ---------------------------------------------------------------------------

import numpy as np

_NC_CACHE = {}


def _get_nc():
    if "nc" not in _NC_CACHE:
        _NC_CACHE["nc"] = build_nc()
    return _NC_CACHE["nc"]


def kernel(x, w1, b1, w2, b2):
    """Channel-attention forward for x:(8,256,128,128); returns same shape.

    Data-parallel over the batch: one batch element per NeuronCore.
    """
    from concourse.bass_utils import run_bass_kernel_spmd

    x = np.asarray(x, dtype=np.float32)
    B, C_, H, W = x.shape
    N = H * W
    nc = _get_nc()

    x16 = np.ascontiguousarray(x.reshape(B, C_, N).astype(np.float16))
    w1t = np.ascontiguousarray(np.asarray(w1, dtype=np.float32).T)
    w2t = np.ascontiguousarray(np.asarray(w2, dtype=np.float32).T)
    b1r = np.ascontiguousarray(np.asarray(b1, dtype=np.float32).reshape(1, C_))
    b2r = np.ascontiguousarray(np.asarray(b2, dtype=np.float32).reshape(1, C_))

    in_maps = [
        {"x": x16[i], "w1t": w1t, "w2t": w2t, "b1": b1r, "b2": b2r}
        for i in range(B)
    ]
    res = run_bass_kernel_spmd(nc, in_maps, core_ids=list(range(B)))
    out = np.stack(
        [res.results[i]["y"].astype(np.float32) for i in range(B)], axis=0
    )
    return out.reshape(B, C_, H, W)
